# revision 27
# baseline (speedup 1.0000x reference)
"""Trainium2 Bass kernel for nn_Attention_558345749040.

Reference computation (per batch b, H=8 heads of d=64, S=4096, E=512):
    Q = Q_seq @ WQ ; K = K_seq @ WK ; V = V_seq @ WV      (per-token matmuls)
    A = (Q * K) / 8                                        (elementwise)
    A += -1e12 at head positions j >= V_len[b]             (additive mask)
    softmax over each head's 64-wide feature group
    O = softmax * V, rows s >= Q_len[b] zeroed

Everything is elementwise across tokens, so rows past Q_len[b] are zero and
are skipped entirely. Active 128-token chunks of every batch are spread
evenly over the 8 cores: batch b contributes m_b = ceil(ceil(Q_len_b/128)/8)
slots per core; every core runs the identical program (slot i belongs to
batch sched[i] on all cores; cores differ only in which token range fills
each slot). Host packs per-core inputs and scatters per-core outputs back.

The V_len mask keeps a prefix j < vl of each head's 64 features, so masked
feature columns are simply NOT COMPUTED: WK/WV are host-packed down to the
FW_b = 8*vl_b live columns per batch, shrinking the K/V matmuls and the
whole softmax chain. WQ stays full/shared; the q*k multiply reads the Q
PSUM through a strided [128, 8, vl] access pattern to pick matching
columns. V_len==0 batches need the reference's uniform softmax V/64: for
those only the (full-width) V projection runs and the host divides by 64.

Device per slot (128 tokens x FW features), matmul IO fp16, PSUM f32
(verified rel-err ~0.004 vs the f32 reference):
  PE:  psk = x_k @ WKp_b, psq = x_q @ WQ (512 wide), psv = x_v @ WVp_b
  ACT: k_sb <- psk copy, exp, v_sb <- psv copy
  DVE: a = psq(strided) * k_sb (f16), group max (negated), group sums
  GP:  t = a - max (broadcast), o = e * v_sb
The softmax division (o/groupsum) and Q_len row masking happen on host:
the device ships unnormalized o (bf16) plus group sums (f32).
"""

import numpy as np
import ml_dtypes

B, S, EMB = 8, 4096, 512
H, D = 8, 64
NCORES = 8
KC = EMB // 128          # 4 contraction chunks
CMAX = S // 128          # 32 chunks per batch max
W = 2                    # slots per wide elementwise tile
SUP = 8                  # slots per input super-DMA (first super is W)

_CACHE = {}


def _schedule(Q_len, V_len):
    """Chunk counts, per-core slots per batch, slot->batch schedule."""
    C = [min(CMAX, -(-int(Q_len[b, 0]) // 128)) for b in range(B)]
    m = [-(-c // NCORES) for c in C]
    vls = {b: int(V_len[b, 0]) for b in range(B)}
    # Heaviest (widest-FW) batches first: early compute-heavy pairs let the
    # input DMA stream build a lead that carries the light-FW tail pairs.
    # V_len==0 batches (V-only, lightest) go last.
    order = sorted(range(B), key=lambda b: (vls[b] == 0, -_fw(vls, b), b))
    sched = []
    off = {}
    for b in order:
        if m[b] == 0:
            continue
        off[b] = len(sched)
        sched += [b] * m[b]
    if len(sched) % W:
        sched.append(sched[-1])  # pad slot: zero inputs, output ignored
    return tuple(sched), C, m, off


def _supers(n_slots):
    out = []
    s = 0
    while s < n_slots:
        take = W if s == 0 else min(SUP, n_slots - s)
        out.append((s, s + take))
        s += take
    return out


def _fw(vls, b):
    # packed feature width per batch; V_len==0 keeps full width (V-only path)
    return 8 * (vls[b] if vls[b] > 0 else D)


def _build(sched, vls):
    import concourse.bacc as bacc
    import concourse.mybir as mybir
    from concourse.tile import TileContext

    f32 = mybir.dt.float32
    f16 = mybir.dt.float16
    bf16 = mybir.dt.bfloat16
    AX = mybir.AxisListType
    OP = mybir.AluOpType
    ACTF = mybir.ActivationFunctionType

    n_slots = len(sched)
    npairs = n_slots // W
    sups = _supers(n_slots)
    sup_starts = {s0: i for i, (s0, s1) in enumerate(sups)}
    batches = sorted(set(sched))

    # packed widths and per-slot column offsets in the paired work tiles
    fw = {b: _fw(vls, b) for b in batches}
    isv0 = {b: vls[b] == 0 for b in batches}
    # weight dram layout: per batch [wk packed (unless v0) | wv packed]
    woff = {}
    wlen = {}
    base = 0
    for b in batches:
        nk = 0 if isv0[b] else KC * fw[b]
        woff[b] = base
        wlen[b] = nk + KC * fw[b]
        base += wlen[b]
    wtot = base

    nc = bacc.Bacc()

    xcols = 12 * 128 * n_slots
    xpack = nc.declare_dram_parameter("xpack", [128, xcols], f16, isOutput=False)
    wq_d = nc.declare_dram_parameter("wq", [128, KC * EMB], f16, isOutput=False)
    wkv_d = nc.declare_dram_parameter("wkv", [128, wtot], f16, isOutput=False)
    out_d = nc.declare_dram_parameter("out", [n_slots * 128, EMB], bf16,
                                      isOutput=True)
    ssum_d = nc.declare_dram_parameter("ssum", [128, 8 * n_slots], f32,
                                       isOutput=True)

    sup_base = {}
    base = 0
    for i, (s0, s1) in enumerate(sups):
        sup_base[i] = base
        base += 12 * 128 * (s1 - s0)

    def xslice(xt, t, s, kc):
        i = 0
        while not (sups[i][0] <= s < sups[i][1]):
            i += 1
        s0, s1 = sups[i]
        ntok = 128 * (s1 - s0)
        off = (t * KC + kc) * ntok + (s - s0) * 128
        return xt[i][:, off:off + 128]

    with TileContext(nc) as tc:
        with (
            tc.tile_pool(name="consts", bufs=1) as cpool,
            tc.tile_pool(name="xin", bufs=2) as xpool,
            tc.tile_pool(name="psk", bufs=2, space="PSUM") as kppool,
            tc.tile_pool(name="psq", bufs=4, space="PSUM") as qppool,
            tc.tile_pool(name="psv", bufs=2, space="PSUM") as vppool,
            tc.tile_pool(name="work", bufs=3) as wpool,
            tc.tile_pool(name="ksb", bufs=4) as kpool,
            tc.tile_pool(name="live", bufs=3) as lpool,
            tc.tile_pool(name="stats", bufs=3) as spool,
        ):
            wq_t = cpool.tile([128, KC * EMB], f16, tag="wq", name="wq")
            wkv_t = {}
            for b in batches:
                wkv_t[b] = cpool.tile([128, wlen[b]], f16, tag=f"wkv{b}",
                                      name=f"wkv{b}")
            ssum_t = cpool.tile([128, 8 * n_slots], f32, tag="ssum", name="ssum")
            xt = [None] * len(sups)

            issued_w = set()

            def ensure_weights(b):
                if b in issued_w:
                    return
                issued_w.add(b)
                # weight + ssum DMAs ride the ACT hardware DGE queue
                nc.scalar.dma_start(out=wkv_t[b][:],
                                    in_=wkv_d[:, woff[b]:woff[b] + wlen[b]])

            def load_super(i):
                s0, s1 = sups[i]
                ntok = 128 * (s1 - s0)
                t = xpool.tile([128, 12 * 128 * SUP], f16, tag="xs", name="xs")
                # split per tensor; k and q (the PE-critical segments) ride
                # the SP queue, v rides the ACT DGE queue: the two hardware
                # queues stream concurrently and the k/q path decongests
                for tens in (1, 0):
                    c0 = tens * KC * ntok
                    nc.sync.dma_start(
                        out=t[:, c0:c0 + KC * ntok],
                        in_=xpack[:, sup_base[i] + c0:sup_base[i] + c0 + KC * ntok],
                    )
                c0 = 2 * KC * ntok
                nc.scalar.dma_start(
                    out=t[:, c0:c0 + KC * ntok],
                    in_=xpack[:, sup_base[i] + c0:sup_base[i] + c0 + KC * ntok],
                )
                xt[i] = t

            ensure_weights(sched[0])
            nc.scalar.dma_start(out=wq_t[:], in_=wq_d[:, :])
            load_super(0)
            for s in range(1, min(4, n_slots)):
                ensure_weights(sched[s])

            def kq_stage(s, c, a, aoff):
                """K+Q matmuls, k copy, a = q*k for one normal slot."""
                b = sched[s]
                w = fw[b]
                vl = vls[b]
                psk = kppool.tile([128, w], f32, tag="psk", name="psk")
                for kc in range(KC):
                    nc.tensor.matmul(
                        psk[:], xslice(xt, 1, s, kc),
                        wkv_t[b][:, kc * w:(kc + 1) * w],
                        start=(kc == 0), stop=(kc == KC - 1),
                    )
                k_sb = kpool.tile([128, w], f16, tag="k_sb", name="k_sb")
                nc.scalar.copy(k_sb[:], psk[:])
                psq = qppool.tile([128, EMB], f32, tag="psq", name="psq")
                for kc in range(KC):
                    nc.tensor.matmul(
                        psq[:], xslice(xt, 0, s, kc),
                        wq_t[:, kc * EMB:(kc + 1) * EMB],
                        start=(kc == 0), stop=(kc == KC - 1),
                    )
                psq_v = psq[:].rearrange("p (g d) -> p g d", d=D)[:, :, :vl]
                nc.vector.tensor_mul(
                    a[:, aoff:aoff + w].rearrange("p (g d) -> p g d", d=vl),
                    psq_v, k_sb[:].rearrange("p (g d) -> p g d", d=vl))

            def v_stage(s, c, dst, doff):
                """V matmuls + psum copy into dst[:, doff:doff+w]."""
                b = sched[s]
                w = fw[b]
                voff = 0 if isv0[b] else KC * w
                psv = vppool.tile([128, w], f32, tag="psv", name="psv")
                for kc in range(KC):
                    nc.tensor.matmul(
                        psv[:], xslice(xt, 2, s, kc),
                        wkv_t[b][:, voff + kc * w:voff + (kc + 1) * w],
                        start=(kc == 0), stop=(kc == KC - 1),
                    )
                nc.scalar.copy(dst[:, doff:doff + w], psv[:])

            def front(pair):
                s0 = pair * W
                if s0 in sup_starts:
                    i = sup_starts[s0]
                    if i + 1 < len(sups):
                        load_super(i + 1)
                for s in range(s0 + 2, min(s0 + 6, n_slots)):
                    ensure_weights(sched[s])

                b0, b1 = sched[s0], sched[s0 + 1]
                offs = [0, fw[b0]]
                wtotal = fw[b0] + fw[b1]
                norm = [c for c in range(W) if not isv0[sched[s0 + c]]]

                a = wpool.tile([128, 1024], f16, tag="a", name="a")
                for c in norm:
                    kq_stage(s0 + c, c, a, offs[c])
                e = None
                if norm:
                    mneg = spool.tile([128, W * H], f16, tag="mneg", name="mneg")
                    t_m = wpool.tile([128, 1024], f16, tag="t_m", name="t_m")
                    if len(norm) == 2 and vls[b0] == vls[b1]:
                        vl = vls[b0]
                        nc.vector.tensor_reduce(
                            mneg[:], a[:, :wtotal].rearrange(
                                "p (g d) -> p g d", d=vl),
                            axis=AX.X, op=OP.max, negate=True)
                        nc.gpsimd.tensor_add(
                            t_m[:, :wtotal].rearrange("p (g d) -> p g d", d=vl),
                            a[:, :wtotal].rearrange("p (g d) -> p g d", d=vl),
                            mneg[:].rearrange("p (g o) -> p g o", o=1)
                            .broadcast_to((128, 2 * H, vl)))
                    else:
                        for c in norm:
                            b = sched[s0 + c]
                            vl = vls[b]
                            w = fw[b]
                            av = a[:, offs[c]:offs[c] + w].rearrange(
                                "p (g d) -> p g d", d=vl)
                            nc.vector.tensor_reduce(
                                mneg[:, c * H:(c + 1) * H], av,
                                axis=AX.X, op=OP.max, negate=True)
                            nc.gpsimd.tensor_add(
                                t_m[:, offs[c]:offs[c] + w].rearrange(
                                    "p (g d) -> p g d", d=vl),
                                av,
                                mneg[:, c * H:(c + 1) * H].rearrange(
                                    "p (g o) -> p g o", o=1)
                                .broadcast_to((128, H, vl)))
                    e = lpool.tile([128, 1024], bf16, tag="e", name="e")
                    if len(norm) == 2:
                        nc.scalar.activation(e[:, :wtotal], t_m[:, :wtotal],
                                             ACTF.Exp)
                    else:
                        c = norm[0]
                        nc.scalar.activation(
                            e[:, offs[c]:offs[c] + fw[sched[s0 + c]]],
                            t_m[:, offs[c]:offs[c] + fw[sched[s0 + c]]],
                            ACTF.Exp)
                v_sb = lpool.tile([128, 1024], bf16, tag="v_sb", name="v_sb")
                for c in range(W):
                    v_stage(s0 + c, c, v_sb, offs[c])
                return e, v_sb

            def back(pair, e, v_sb):
                s0 = pair * W
                b0, b1 = sched[s0], sched[s0 + 1]
                offs = [0, fw[b0]]
                wtotal = fw[b0] + fw[b1]
                norm = [c for c in range(W) if not isv0[sched[s0 + c]]]

                for c in norm:
                    b = sched[s0 + c]
                    vl = vls[b]
                    w = fw[b]
                    nc.vector.tensor_reduce(
                        ssum_t[:, (s0 + c) * H:(s0 + c + 1) * H],
                        e[:, offs[c]:offs[c] + w].rearrange(
                            "p (g d) -> p g d", d=vl),
                        axis=AX.X, op=OP.add)
                o = wpool.tile([128, 1024], bf16, tag="o", name="o")
                if len(norm) == 2:
                    nc.gpsimd.tensor_mul(o[:, :wtotal], e[:, :wtotal],
                                         v_sb[:, :wtotal])
                elif norm:
                    c = norm[0]
                    w = fw[sched[s0 + c]]
                    nc.gpsimd.tensor_mul(o[:, offs[c]:offs[c] + w],
                                         e[:, offs[c]:offs[c] + w],
                                         v_sb[:, offs[c]:offs[c] + w])
                for c in range(W):
                    s = s0 + c
                    b = sched[s]
                    w = fw[b]
                    src = v_sb if isv0[b] else o
                    nc.sync.dma_start(
                        out=out_d[s * 128:(s + 1) * 128, :w],
                        in_=src[:, offs[c]:offs[c] + w],
                    )

            pending = None
            for pair in range(npairs + 1):
                fr = front(pair) if pair < npairs else None
                if pending is not None:
                    back(pair - 1, *pending)
                pending = fr

            nc.scalar.dma_start(out=ssum_d[:, :], in_=ssum_t[:])

    nc.finalize()
    return nc


def _prep_inputs(Q_seq, K_seq, V_seq, Q_len, V_len, WQ, WK, WV, sched, C, m):
    n_slots = len(sched)
    sups = _supers(n_slots)
    batches = sorted(set(sched))
    vls = {b: int(V_len[b, 0]) for b in batches}
    fw = {b: _fw(vls, b) for b in batches}
    off = {}
    for s, b in enumerate(sched):
        if b not in off:
            off[b] = s

    xt16 = {}
    for b in batches:
        xt16[b] = [
            np.ascontiguousarray(np.asarray(t[b]).T.astype(np.float16))
            for t in (Q_seq, K_seq, V_seq)
        ]

    def wpack(wmat):
        # [512, FWcols] -> [128, 4*FW] kc-blocks
        fwc = wmat.shape[1]
        return np.ascontiguousarray(
            wmat.reshape(KC, 128, fwc).transpose(1, 0, 2).reshape(128, KC * fwc)
        )

    wq_h = wpack((np.asarray(WQ) * 0.125).astype(np.float16))

    wparts = []
    for b in batches:
        vl = vls[b]
        if vl == 0:
            wparts.append(wpack(np.asarray(WV).astype(np.float16)))
        else:
            idx = (np.arange(H)[:, None] * D + np.arange(vl)[None, :]).ravel()
            wparts.append(wpack(np.asarray(WK)[:, idx].astype(np.float16)))
            wparts.append(wpack(np.asarray(WV)[:, idx].astype(np.float16)))
    wkv_h = np.concatenate(wparts, axis=1)

    in_maps = []
    for c in range(NCORES):
        xp = np.zeros((128, 12 * 128 * n_slots), np.float16)
        base = 0
        for (s0, s1) in sups:
            ntok = 128 * (s1 - s0)
            for t in range(3):
                for kc in range(KC):
                    for s in range(s0, s1):
                        b = sched[s]
                        chunk = c * m[b] + (s - off[b])
                        if chunk >= C[b]:
                            continue
                        col = base + (t * KC + kc) * ntok + (s - s0) * 128
                        xp[:, col:col + 128] = \
                            xt16[b][t][kc * 128:(kc + 1) * 128,
                                       chunk * 128:(chunk + 1) * 128]
            base += 12 * ntok
        in_maps.append({
            "xpack": xp,
            "wq": wq_h,
            "wkv": wkv_h,
        })
    return in_maps, off


def _postprocess(results, Q_len, V_len, sched, C, m, off):
    vls = {b: int(V_len[b, 0]) for b in set(sched)}
    outf = np.zeros((B, S, EMB), np.float32)
    for c in range(NCORES):
        o_un = results[c]["out"].astype(np.float32)
        ss = results[c]["ssum"].astype(np.float32)
        for b in sorted(set(sched)):
            vl = vls[b]
            w = _fw(vls, b)
            for j in range(m[b]):
                chunk = c * m[b] + j
                if chunk >= C[b]:
                    continue
                s = off[b] + j
                rows = min(128, int(Q_len[b, 0]) - chunk * 128)
                if rows <= 0:
                    continue
                blk = o_un[s * 128:s * 128 + rows, :w]
                dst = outf[b, chunk * 128:chunk * 128 + rows].reshape(
                    rows, H, D)
                if vl == 0:
                    dst[:, :, :] = blk.reshape(rows, H, D) / 64.0
                else:
                    sc = ss[:rows, s * H:(s + 1) * H]
                    denom = np.where(sc > 0, sc, 1.0)
                    dst[:, :, :vl] = \
                        blk.reshape(rows, H, vl) / denom[:, :, None]
    return outf


def _run(inputs, trace=False, mm_dtype_name="", tmpdir=None):
    from concourse.bass_utils import run_bass_kernel_spmd

    Q_len = np.asarray(inputs["Q_len"])
    V_len = np.asarray(inputs["V_len"])
    sched, C, m, _ = _schedule(Q_len, V_len)
    if not sched:
        return np.zeros((B, S, EMB), np.float32), None

    vls = {b: int(V_len[b, 0]) for b in set(sched)}
    key = ("v23", sched, tuple(sorted(vls.items())))
    if key not in _CACHE:
        _CACHE[key] = _build(sched, vls)
    nc = _CACHE[key]

    in_maps, off = _prep_inputs(
        np.asarray(inputs["Q_seq"]), np.asarray(inputs["K_seq"]),
        np.asarray(inputs["V_seq"]), Q_len, V_len,
        np.asarray(inputs["WQ"]), np.asarray(inputs["WK"]),
        np.asarray(inputs["WV"]), sched, C, m)
    res = run_bass_kernel_spmd(nc, in_maps, core_ids=list(range(NCORES)),
                               trace=trace, tmpdir=tmpdir)
    out = _postprocess(res.results, Q_len, V_len, sched, C, m, off)
    return out, res


def kernel(Q_seq, K_seq, V_seq, Q_len, V_len, WQ, WK, WV):
    out, _ = _run(dict(Q_seq=Q_seq, K_seq=K_seq, V_seq=V_seq,
                       Q_len=Q_len, V_len=V_len, WQ=WQ, WK=WK, WV=WV))
    return out


# revision 29
# speedup vs baseline: 1.0707x; 1.0707x over previous
"""Trainium2 Bass kernel for nn_Attention_558345749040.

Reference computation (per batch b, H=8 heads of d=64, S=4096, E=512):
    Q = Q_seq @ WQ ; K = K_seq @ WK ; V = V_seq @ WV      (per-token matmuls)
    A = (Q * K) / 8                                        (elementwise)
    A += -1e12 at head positions j >= V_len[b]             (additive mask)
    softmax over each head's 64-wide feature group
    O = softmax * V, rows s >= Q_len[b] zeroed

Everything is elementwise across tokens, so rows past Q_len[b] are zero and
are skipped entirely. Active 128-token chunks of every batch are spread
evenly over the 8 cores: batch b contributes m_b = ceil(ceil(Q_len_b/128)/8)
slots per core; every core runs the identical program (slot i belongs to
batch sched[i] on all cores; cores differ only in which token range fills
each slot). Host packs per-core inputs and scatters per-core outputs back.

The V_len mask keeps a prefix j < vl of each head's 64 features, so masked
feature columns are simply NOT COMPUTED: WK/WV are host-packed down to the
FW_b = 8*vl_b live columns per batch, shrinking the K/V matmuls and the
whole softmax chain. WQ stays full/shared; the q*k multiply reads the Q
PSUM through a strided [128, 8, vl] access pattern to pick matching
columns. V_len==0 batches need the reference's uniform softmax V/64: for
those only the (full-width) V projection runs and the host divides by 64.

Device per slot (128 tokens x FW features), matmul IO fp16, PSUM f32
(verified rel-err ~0.004 vs the f32 reference):
  PE:  psk = x_k @ WKp_b, psq = x_q @ WQ (512 wide), psv = x_v @ WVp_b
  ACT: k_sb <- psk copy, exp, v_sb <- psv copy
  DVE: a = psq(strided) * k_sb (f16), group max (negated), group sums
  GP:  t = a - max (broadcast), o = e * v_sb
The softmax division (o/groupsum) and Q_len row masking happen on host:
the device ships unnormalized o (bf16) plus group sums (f32).
"""

import numpy as np
import ml_dtypes

B, S, EMB = 8, 4096, 512
H, D = 8, 64
NCORES = 8
KC = EMB // 128          # 4 contraction chunks
CMAX = S // 128          # 32 chunks per batch max
W = 2                    # slots per wide elementwise tile
SUP = 8                  # slots per input super-DMA (first super is W)

_CACHE = {}


def _schedule(Q_len, V_len):
    """Chunk counts, per-core slots per batch, slot->batch schedule."""
    C = [min(CMAX, -(-int(Q_len[b, 0]) // 128)) for b in range(B)]
    m = [-(-c // NCORES) for c in C]
    vls = {b: int(V_len[b, 0]) for b in range(B)}
    # Heaviest (widest-FW) batches first: early compute-heavy pairs let the
    # input DMA stream build a lead that carries the light-FW tail pairs.
    # V_len==0 batches (V-only, lightest) go last.
    order = sorted(range(B), key=lambda b: (vls[b] == 0, -_fw(vls, b), b))
    sched = []
    off = {}
    for b in order:
        if m[b] == 0:
            continue
        off[b] = len(sched)
        sched += [b] * m[b]
    if len(sched) % W:
        sched.append(sched[-1])  # pad slot: zero inputs, output ignored
    return tuple(sched), C, m, off


def _supers(n_slots):
    out = []
    s = 0
    while s < n_slots:
        take = W if s == 0 else min(SUP, n_slots - s)
        out.append((s, s + take))
        s += take
    return out


def _fw(vls, b):
    # packed feature width per batch; V_len==0 keeps full width (V-only path)
    return 8 * (vls[b] if vls[b] > 0 else D)


def _build(sched, vls):
    import concourse.bacc as bacc
    import concourse.mybir as mybir
    from concourse.tile import TileContext

    f32 = mybir.dt.float32
    f16 = mybir.dt.float16
    bf16 = mybir.dt.bfloat16
    AX = mybir.AxisListType
    OP = mybir.AluOpType
    ACTF = mybir.ActivationFunctionType

    n_slots = len(sched)
    npairs = n_slots // W
    sups = _supers(n_slots)
    sup_starts = {s0: i for i, (s0, s1) in enumerate(sups)}
    batches = sorted(set(sched))

    # packed widths and per-slot column offsets in the paired work tiles
    fw = {b: _fw(vls, b) for b in batches}
    isv0 = {b: vls[b] == 0 for b in batches}
    # weight dram layout: per batch [wk packed (unless v0) | wv packed]
    woff = {}
    wlen = {}
    base = 0
    for b in batches:
        nk = 0 if isv0[b] else KC * fw[b]
        woff[b] = base
        wlen[b] = nk + KC * fw[b]
        base += wlen[b]
    wtot = base

    nc = bacc.Bacc()

    xcols = 12 * 128 * n_slots
    xpack = nc.declare_dram_parameter("xpack", [128, xcols], f16, isOutput=False)
    wq_d = nc.declare_dram_parameter("wq", [128, KC * EMB], f16, isOutput=False)
    wkv_d = nc.declare_dram_parameter("wkv", [128, wtot], f16, isOutput=False)
    out_d = nc.declare_dram_parameter("out", [n_slots * 128, EMB], bf16,
                                      isOutput=True)
    ssum_d = nc.declare_dram_parameter("ssum", [128, 8 * n_slots], f32,
                                       isOutput=True)

    sup_base = {}
    base = 0
    for i, (s0, s1) in enumerate(sups):
        sup_base[i] = base
        base += 12 * 128 * (s1 - s0)

    def xslice(xt, t, s, kc):
        i = 0
        while not (sups[i][0] <= s < sups[i][1]):
            i += 1
        s0, s1 = sups[i]
        ntok = 128 * (s1 - s0)
        off = (t * KC + kc) * ntok + (s - s0) * 128
        return xt[i][:, off:off + 128]

    with TileContext(nc) as tc:
        with (
            tc.tile_pool(name="consts", bufs=1) as cpool,
            tc.tile_pool(name="xin", bufs=2) as xpool,
            tc.tile_pool(name="psk", bufs=2, space="PSUM") as kppool,
            tc.tile_pool(name="psq", bufs=4, space="PSUM") as qppool,
            tc.tile_pool(name="psv", bufs=2, space="PSUM") as vppool,
            tc.tile_pool(name="work", bufs=3) as wpool,
            tc.tile_pool(name="ksb", bufs=4) as kpool,
            tc.tile_pool(name="live", bufs=3) as lpool,
            tc.tile_pool(name="stats", bufs=3) as spool,
        ):
            wq_t = cpool.tile([128, KC * EMB], f16, tag="wq", name="wq")
            wkv_t = {}
            for b in batches:
                wkv_t[b] = cpool.tile([128, wlen[b]], f16, tag=f"wkv{b}",
                                      name=f"wkv{b}")
            ssum_t = cpool.tile([128, 8 * n_slots], f32, tag="ssum", name="ssum")
            xt = [None] * len(sups)

            issued_w = set()

            def ensure_weights(b):
                if b in issued_w:
                    return
                issued_w.add(b)
                # weight + ssum DMAs ride the ACT hardware DGE queue
                nc.scalar.dma_start(out=wkv_t[b][:],
                                    in_=wkv_d[:, woff[b]:woff[b] + wlen[b]])

            def load_super(i):
                s0, s1 = sups[i]
                ntok = 128 * (s1 - s0)
                t = xpool.tile([128, 12 * 128 * SUP], f16, tag="xs", name="xs")
                # split per tensor; k and q (the PE-critical segments) ride
                # the SP queue, v rides the ACT DGE queue: the two hardware
                # queues stream concurrently and the k/q path decongests
                for tens in (1, 0):
                    c0 = tens * KC * ntok
                    nc.sync.dma_start(
                        out=t[:, c0:c0 + KC * ntok],
                        in_=xpack[:, sup_base[i] + c0:sup_base[i] + c0 + KC * ntok],
                    )
                c0 = 2 * KC * ntok
                nc.scalar.dma_start(
                    out=t[:, c0:c0 + KC * ntok],
                    in_=xpack[:, sup_base[i] + c0:sup_base[i] + c0 + KC * ntok],
                )
                xt[i] = t

            ensure_weights(sched[0])
            nc.scalar.dma_start(out=wq_t[:], in_=wq_d[:, :])
            load_super(0)
            for s in range(1, min(4, n_slots)):
                ensure_weights(sched[s])

            def kq_stage(s, c, a, aoff):
                """K+Q matmuls, k copy, a = q*k for one normal slot."""
                b = sched[s]
                w = fw[b]
                vl = vls[b]
                psk = kppool.tile([128, w], f32, tag="psk", name="psk")
                for kc in range(KC):
                    nc.tensor.matmul(
                        psk[:], xslice(xt, 1, s, kc),
                        wkv_t[b][:, kc * w:(kc + 1) * w],
                        start=(kc == 0), stop=(kc == KC - 1),
                    )
                k_sb = kpool.tile([128, w], f16, tag="k_sb", name="k_sb")
                nc.scalar.copy(k_sb[:], psk[:])
                psq = qppool.tile([128, EMB], f32, tag="psq", name="psq")
                for kc in range(KC):
                    nc.tensor.matmul(
                        psq[:], xslice(xt, 0, s, kc),
                        wq_t[:, kc * EMB:(kc + 1) * EMB],
                        start=(kc == 0), stop=(kc == KC - 1),
                    )
                psq_v = psq[:].rearrange("p (g d) -> p g d", d=D)[:, :, :vl]
                nc.vector.tensor_mul(
                    a[:, aoff:aoff + w].rearrange("p (g d) -> p g d", d=vl),
                    psq_v, k_sb[:].rearrange("p (g d) -> p g d", d=vl))

            def v_stage(s, c, dst, doff):
                """V matmuls + psum copy into dst[:, doff:doff+w]."""
                b = sched[s]
                w = fw[b]
                voff = 0 if isv0[b] else KC * w
                psv = vppool.tile([128, w], f32, tag="psv", name="psv")
                for kc in range(KC):
                    nc.tensor.matmul(
                        psv[:], xslice(xt, 2, s, kc),
                        wkv_t[b][:, voff + kc * w:voff + (kc + 1) * w],
                        start=(kc == 0), stop=(kc == KC - 1),
                    )
                nc.scalar.copy(dst[:, doff:doff + w], psv[:])

            def front(pair):
                s0 = pair * W
                if s0 in sup_starts:
                    i = sup_starts[s0]
                    if i + 1 < len(sups):
                        load_super(i + 1)
                for s in range(s0 + 2, min(s0 + 6, n_slots)):
                    ensure_weights(sched[s])

                b0, b1 = sched[s0], sched[s0 + 1]
                offs = [0, fw[b0]]
                wtotal = fw[b0] + fw[b1]
                norm = [c for c in range(W) if not isv0[sched[s0 + c]]]

                a = wpool.tile([128, 1024], f16, tag="a", name="a")
                for c in norm:
                    kq_stage(s0 + c, c, a, offs[c])
                e = None
                if norm:
                    mneg = spool.tile([128, W * H], f16, tag="mneg", name="mneg")
                    t_m = wpool.tile([128, 1024], f16, tag="t_m", name="t_m")
                    if len(norm) == 2 and vls[b0] == vls[b1]:
                        vl = vls[b0]
                        nc.vector.tensor_reduce(
                            mneg[:], a[:, :wtotal].rearrange(
                                "p (g d) -> p g d", d=vl),
                            axis=AX.X, op=OP.max, negate=True)
                        nc.gpsimd.tensor_add(
                            t_m[:, :wtotal].rearrange("p (g d) -> p g d", d=vl),
                            a[:, :wtotal].rearrange("p (g d) -> p g d", d=vl),
                            mneg[:].rearrange("p (g o) -> p g o", o=1)
                            .broadcast_to((128, 2 * H, vl)))
                    else:
                        for c in norm:
                            b = sched[s0 + c]
                            vl = vls[b]
                            w = fw[b]
                            av = a[:, offs[c]:offs[c] + w].rearrange(
                                "p (g d) -> p g d", d=vl)
                            nc.vector.tensor_reduce(
                                mneg[:, c * H:(c + 1) * H], av,
                                axis=AX.X, op=OP.max, negate=True)
                            nc.gpsimd.tensor_add(
                                t_m[:, offs[c]:offs[c] + w].rearrange(
                                    "p (g d) -> p g d", d=vl),
                                av,
                                mneg[:, c * H:(c + 1) * H].rearrange(
                                    "p (g o) -> p g o", o=1)
                                .broadcast_to((128, H, vl)))
                    e = lpool.tile([128, 1024], bf16, tag="e", name="e")
                    if len(norm) == 2:
                        nc.scalar.activation(e[:, :wtotal], t_m[:, :wtotal],
                                             ACTF.Exp)
                    else:
                        c = norm[0]
                        nc.scalar.activation(
                            e[:, offs[c]:offs[c] + fw[sched[s0 + c]]],
                            t_m[:, offs[c]:offs[c] + fw[sched[s0 + c]]],
                            ACTF.Exp)
                v_sb = lpool.tile([128, 1024], bf16, tag="v_sb", name="v_sb")
                for c in range(W):
                    v_stage(s0 + c, c, v_sb, offs[c])
                return e, v_sb

            def back(pair, e, v_sb):
                s0 = pair * W
                b0, b1 = sched[s0], sched[s0 + 1]
                offs = [0, fw[b0]]
                wtotal = fw[b0] + fw[b1]
                norm = [c for c in range(W) if not isv0[sched[s0 + c]]]

                for c in norm:
                    b = sched[s0 + c]
                    vl = vls[b]
                    w = fw[b]
                    nc.vector.tensor_reduce(
                        ssum_t[:, (s0 + c) * H:(s0 + c + 1) * H],
                        e[:, offs[c]:offs[c] + w].rearrange(
                            "p (g d) -> p g d", d=vl),
                        axis=AX.X, op=OP.add)
                o = wpool.tile([128, 1024], bf16, tag="o", name="o")
                if len(norm) == 2:
                    nc.gpsimd.tensor_mul(o[:, :wtotal], e[:, :wtotal],
                                         v_sb[:, :wtotal])
                elif norm:
                    c = norm[0]
                    w = fw[sched[s0 + c]]
                    nc.gpsimd.tensor_mul(o[:, offs[c]:offs[c] + w],
                                         e[:, offs[c]:offs[c] + w],
                                         v_sb[:, offs[c]:offs[c] + w])
                for c in range(W):
                    s = s0 + c
                    b = sched[s]
                    w = fw[b]
                    src = v_sb if isv0[b] else o
                    nc.sync.dma_start(
                        out=out_d[s * 128:(s + 1) * 128, :w],
                        in_=src[:, offs[c]:offs[c] + w],
                    )

            pending = None
            for pair in range(npairs + 1):
                fr = front(pair) if pair < npairs else None
                if pending is not None:
                    back(pair - 1, *pending)
                pending = fr

            nc.scalar.dma_start(out=ssum_d[:, :], in_=ssum_t[:])

    nc.finalize()
    return nc


def _prep_inputs(Q_seq, K_seq, V_seq, Q_len, V_len, WQ, WK, WV, sched, C, m):
    n_slots = len(sched)
    sups = _supers(n_slots)
    batches = sorted(set(sched))
    vls = {b: int(V_len[b, 0]) for b in batches}
    fw = {b: _fw(vls, b) for b in batches}
    off = {}
    for s, b in enumerate(sched):
        if b not in off:
            off[b] = s

    xt16 = {}
    for b in batches:
        xt16[b] = [
            np.ascontiguousarray(np.asarray(t[b]).T.astype(np.float16))
            for t in (Q_seq, K_seq, V_seq)
        ]

    def wpack(wmat):
        # [512, FWcols] -> [128, 4*FW] kc-blocks
        fwc = wmat.shape[1]
        return np.ascontiguousarray(
            wmat.reshape(KC, 128, fwc).transpose(1, 0, 2).reshape(128, KC * fwc)
        )

    wq_h = wpack((np.asarray(WQ) * 0.125).astype(np.float16))

    wparts = []
    for b in batches:
        vl = vls[b]
        if vl == 0:
            wparts.append(wpack(np.asarray(WV).astype(np.float16)))
        else:
            idx = (np.arange(H)[:, None] * D + np.arange(vl)[None, :]).ravel()
            wparts.append(wpack(np.asarray(WK)[:, idx].astype(np.float16)))
            wparts.append(wpack(np.asarray(WV)[:, idx].astype(np.float16)))
    wkv_h = np.concatenate(wparts, axis=1)

    in_maps = []
    for c in range(NCORES):
        xp = np.zeros((128, 12 * 128 * n_slots), np.float16)
        base = 0
        for (s0, s1) in sups:
            ntok = 128 * (s1 - s0)
            for t in range(3):
                for kc in range(KC):
                    for s in range(s0, s1):
                        b = sched[s]
                        chunk = c * m[b] + (s - off[b])
                        if chunk >= C[b]:
                            continue
                        col = base + (t * KC + kc) * ntok + (s - s0) * 128
                        xp[:, col:col + 128] = \
                            xt16[b][t][kc * 128:(kc + 1) * 128,
                                       chunk * 128:(chunk + 1) * 128]
            base += 12 * ntok
        in_maps.append({
            "xpack": xp,
            "wq": wq_h,
            "wkv": wkv_h,
        })
    return in_maps, off


def _postprocess(results, Q_len, V_len, sched, C, m, off):
    vls = {b: int(V_len[b, 0]) for b in set(sched)}
    outf = np.zeros((B, S, EMB), np.float32)
    for c in range(NCORES):
        o_un = results[c]["out"].astype(np.float32)
        ss = results[c]["ssum"].astype(np.float32)
        for b in sorted(set(sched)):
            vl = vls[b]
            w = _fw(vls, b)
            for j in range(m[b]):
                chunk = c * m[b] + j
                if chunk >= C[b]:
                    continue
                s = off[b] + j
                rows = min(128, int(Q_len[b, 0]) - chunk * 128)
                if rows <= 0:
                    continue
                blk = o_un[s * 128:s * 128 + rows, :w]
                dst = outf[b, chunk * 128:chunk * 128 + rows].reshape(
                    rows, H, D)
                if vl == 0:
                    dst[:, :, :] = blk.reshape(rows, H, D) / 64.0
                else:
                    sc = ss[:rows, s * H:(s + 1) * H]
                    denom = np.where(sc > 0, sc, 1.0)
                    dst[:, :, :vl] = \
                        blk.reshape(rows, H, vl) / denom[:, :, None]
    return outf


def _run(inputs, trace=False, mm_dtype_name="", tmpdir=None):
    from concourse.bass_utils import run_bass_kernel_spmd

    Q_len = np.asarray(inputs["Q_len"])
    V_len = np.asarray(inputs["V_len"])
    sched, C, m, _ = _schedule(Q_len, V_len)
    if not sched:
        return np.zeros((B, S, EMB), np.float32), None

    vls = {b: int(V_len[b, 0]) for b in set(sched)}
    key = ("v23", sched, tuple(sorted(vls.items())))
    if key not in _CACHE:
        _CACHE[key] = _build(sched, vls)
    nc = _CACHE[key]

    in_maps, off = _prep_inputs(
        np.asarray(inputs["Q_seq"]), np.asarray(inputs["K_seq"]),
        np.asarray(inputs["V_seq"]), Q_len, V_len,
        np.asarray(inputs["WQ"]), np.asarray(inputs["WK"]),
        np.asarray(inputs["WV"]), sched, C, m)
    res = run_bass_kernel_spmd(nc, in_maps, core_ids=list(range(NCORES)),
                               trace=trace, tmpdir=tmpdir)
    out = _postprocess(res.results, Q_len, V_len, sched, C, m, off)
    return out, res


def kernel(Q_seq, K_seq, V_seq, Q_len, V_len, WQ, WK, WV):
    out, _ = _run(dict(Q_seq=Q_seq, K_seq=K_seq, V_seq=V_seq,
                       Q_len=Q_len, V_len=V_len, WQ=WQ, WK=WK, WV=WV))
    return out
